# revision 23
# baseline (speedup 1.0000x reference)
"""Distributed multi-head attention for Trainium2 (8 NeuronCores).

Problem: nn_MultiHeadAttention (B=2, S=2048, D=1024, H=16, HD=64), f32.

Sharding: tensor parallel over heads — core c owns heads {2c, 2c+1}
(feature slice [128c, 128c+128)) and processes BOTH batches for them.
The output projection is sequence-parallel: an 8-core AllToAll exchanges
token blocks of the per-head attention outputs, after which core c holds
all 16 heads for tokens [512c, 512c+512) of the combined (batch, seq)
axis and contracts the full 1024 attention features against Wo.

Matmuls run in bf16 (f32 PSUM accumulate). Key Trainium2 facts shaping
the implementation (HW-measured here):
  - K=64 matmuls stream at ~2 cyc/col vs 1 for K=128, so the scores
    matmuls use per-head zero-padded KT tiles (K=128, zeros kill the
    other head's contribution; QT needs no masking).
  - Every sync-engine dma_start costs ~0.6us of sequencer time, so bulk
    loads are single strided DMAs ([128, 8, 512] etc.), not per-chunk.
  - ScalarE does ONLY exp (switching activation functions reloads LUTs);
    all PSUM evacuations go through VectorE with fused bias/cast.
  - exp is done on [128, 1024] tiles (2 PSUM banks) to amortize ~250ns
    of per-instruction ACT overhead.
  - attn^T = V_aug.T @ exp accumulated over k tiles, where V_aug carries
    a ones column -> psum row 64 is the softmax denominator for free.
  - No max subtraction in softmax: scores ~ N(0,1) by construction.
"""

import numpy as np

B = 2
S = 2048          # both n_q and k (per batch)
TS = B * S        # combined token axis (4096)
D = 1024          # embed dim
H = 16            # heads
HD = 64           # head dim
N_CORES = 8
GH = 2            # heads per core
GF = GH * HD      # 128 per-core head features
TB = 512          # token block (a2a chunk + per-core output slice)
NKT = S // 128    # 16 k tiles per batch
ECH = D // 128    # 8 contraction chunks of the embed dim

_CACHE = {}
MM_DTYPE = "bf16"  # "bf16" or "f32r"


def _build():
    import concourse.bacc as bacc
    import concourse.tile as tile
    from concourse import mybir

    F32 = mybir.dt.float32
    MDT = mybir.dt.bfloat16 if MM_DTYPE == "bf16" else mybir.dt.float32r
    Act = mybir.ActivationFunctionType

    nc = bacc.Bacc("TRN2", target_bir_lowering=False, debug=False,
                   num_devices=N_CORES)

    # ---- kernel I/O ----
    xqT = nc.dram_tensor("xqT", [D, TS], MDT, kind="ExternalInput")
    xkT = nc.dram_tensor("xkT", [D, TS], MDT, kind="ExternalInput")
    xvT = nc.dram_tensor("xvT", [D, TS], MDT, kind="ExternalInput")
    wqT = nc.dram_tensor("wqT", [128, ECH, GF], MDT, kind="ExternalInput")
    wkT = nc.dram_tensor("wkT", [128, ECH, GF], MDT, kind="ExternalInput")
    wvT = nc.dram_tensor("wvT", [128, ECH, GF], MDT, kind="ExternalInput")
    woT = nc.dram_tensor("woT", [128, ECH, D], MDT, kind="ExternalInput")

    bq_d = nc.dram_tensor("bq", [128, 1], F32, kind="ExternalInput")
    bk_d = nc.dram_tensor("bk", [128, 1], F32, kind="ExternalInput")
    kmask_d = nc.dram_tensor("kmask", [128, GH], F32, kind="ExternalInput")
    kbm_d = nc.dram_tensor("kbm", [128, GH], F32, kind="ExternalInput")
    bv_d = nc.dram_tensor("bv", [128, TB], F32, kind="ExternalInput")
    bo_d = nc.dram_tensor("bo", [128, D], F32, kind="ExternalInput")
    out_d = nc.dram_tensor("out", [TB, D], F32, kind="ExternalOutput")

    groups = [list(range(N_CORES))]

    with tile.TileContext(nc) as tc:
        with (
            tc.tile_pool(name="wpool", bufs=1) as wpool,
            tc.tile_pool(name="state", bufs=1) as state,
            tc.tile_pool(name="xpool", bufs=3) as xpool,
            tc.tile_pool(name="expp", bufs=3) as expp,
            tc.tile_pool(name="small", bufs=3) as small,
            tc.tile_pool(name="opool", bufs=2) as opool,
            tc.tile_pool(name="ps_proj", bufs=2, space="PSUM") as ps_proj,
            tc.tile_pool(name="ps_sc", bufs=2, space="PSUM") as ps_sc,
            tc.tile_pool(name="ps_at", bufs=2, space="PSUM") as ps_at,
            tc.tile_pool(name="dramp", bufs=1, space="DRAM") as dramp,
        ):
            # ---- first token block's X tiles, then weights (startup path) ----
            pre_x = {}
            for name, xsrc in (("q", xqT), ("k", xkT), ("v", xvT)):
                xt0 = xpool.tile([128, ECH, TB], MDT, tag="x",
                                 name=f"x{name}pre")
                nc.sync.dma_start(
                    xt0[:], xsrc[:, 0:TB].rearrange("(e p) n -> p e n", p=128))
                pre_x[name] = xt0
            wq_sb = wpool.tile([128, ECH, GF], MDT, name="wq_sb")
            nc.sync.dma_start(wq_sb[:], wqT[:])
            wk_sb = wpool.tile([128, ECH, GF], MDT, name="wk_sb")
            nc.sync.dma_start(wk_sb[:], wkT[:])
            wv_sb = wpool.tile([128, ECH, GF], MDT, name="wv_sb")
            nc.sync.dma_start(wv_sb[:], wvT[:])
            wo_sb = wpool.tile([128, ECH, D], MDT, name="wo_sb")
            bq_sb = wpool.tile([128, 1], F32, name="bq_sb")
            nc.sync.dma_start(bq_sb[:], bq_d[:])
            bk_sb = wpool.tile([128, 1], F32, name="bk_sb")
            nc.sync.dma_start(bk_sb[:], bk_d[:])
            kmask_sb = wpool.tile([128, GH], F32, name="kmask_sb")
            nc.sync.dma_start(kmask_sb[:], kmask_d[:])
            kbm_sb = wpool.tile([128, GH], F32, name="kbm_sb")
            nc.sync.dma_start(kbm_sb[:], kbm_d[:])
            bv_sb = wpool.tile([128, TB], F32, name="bv_sb")
            nc.sync.dma_start(bv_sb[:], bv_d[:])
            bo_sb = wpool.tile([128, D], F32, name="bo_sb")

            # ---- long-lived state ----
            QT = state.tile([128, TS], MDT, name="QT")
            AT = state.tile([128, TS], MDT, name="AT")
            # per-head zero-padded KT: rows [64h, 64h+64) hold head h's
            # K features, the other 64 rows stay zero -> scores matmuls
            # run K=128 (2x faster than K=64) with unmasked QT as rhs.
            KTp = [state.tile([128, TS], MDT, name=f"KTp{h}")
                   for h in range(GH)]

            # V: [128 tok, tok-chunk, head, 65]; col 64 = ones
            VT = state.tile([128, B * NKT, GH, HD + 1], MDT, name="VT")
            nc.gpsimd.memset(VT[:, :, :, HD:HD + 1], 1.0)

            # ---- emission helpers (PE stream order == emission order) ----
            def emit_proj(b):
                for t in range(S // TB):
                    col = b * S + t * TB
                    csl = slice(col, col + TB)
                    # Q, K -> feature-major; K lands in per-head padded rows
                    for name, xsrc, w_sb, b_sb in (
                        ("q", xqT, wq_sb, bq_sb),
                        ("k", xkT, wk_sb, bk_sb),
                    ):
                        if b == 0 and t == 0:
                            xt = pre_x[name]
                        else:
                            xt = xpool.tile([128, ECH, TB], MDT, tag="x",
                                            name=f"x{name}{b}{t}")
                            nc.sync.dma_start(
                                xt[:],
                                xsrc[:, csl].rearrange("(e p) n -> p e n",
                                                       p=128))
                        ps = ps_proj.tile([128, TB], F32, tag="pp",
                                          name=f"ps{name}{b}{t}")
                        for e in range(ECH):
                            nc.tensor.matmul(ps[:], w_sb[:, e, :],
                                             xt[:, e, :],
                                             start=(e == 0),
                                             stop=(e == ECH - 1))
                        if name == "q":
                            nc.vector.tensor_scalar_add(QT[:, csl], ps[:],
                                                        b_sb[:])
                        else:
                            for h in range(GH):
                                nc.vector.tensor_scalar(
                                    KTp[h][:, csl], ps[:],
                                    kmask_sb[:, h:h + 1], kbm_sb[:, h:h + 1],
                                    op0=mybir.AluOpType.mult,
                                    op1=mybir.AluOpType.add)
                    # V -> token-major (4 chunks of 128 tokens share 1 psum)
                    if b == 0 and t == 0:
                        xt = pre_x["v"]
                    else:
                        xt = xpool.tile([128, ECH, TB], MDT, tag="x",
                                        name=f"xv{b}{t}")
                        nc.sync.dma_start(
                            xt[:],
                            xvT[:, csl].rearrange("(e p) n -> p e n", p=128))
                    psv = ps_proj.tile([128, TB], F32, tag="pp",
                                       name=f"psv{b}{t}")
                    for e in range(ECH):
                        for m in range(4):
                            # NOTE: start=True clears has_written for the
                            # WHOLE psum bank, so only the very first matmul
                            # into this bank may set it.
                            nc.tensor.matmul(
                                psv[:, m * GF:(m + 1) * GF],
                                xt[:, e, m * 128:(m + 1) * 128],
                                wv_sb[:, e, :],
                                start=(e == 0 and m == 0),
                                stop=(e == ECH - 1 and m == 3))
                    kt0 = b * NKT + t * 4
                    nc.vector.tensor_add(
                        VT[:, kt0:kt0 + 4, :, 0:HD],
                        psv[:].rearrange("p (m h d) -> p m h d", m=4, h=GH),
                        bv_sb[:].rearrange("p (m h d) -> p m h d", m=4, h=GH))

            # Two collectives, one per head-parity row range of AT: the
            # first launches after head 0's attention and overlaps the
            # rest; each carries rows [64h, 64h+64) for all 8 chunks.
            NP = NKT // 2  # k-tile pairs (wide 1024-col exp tiles)
            a2a_in = [dramp.tile([N_CORES, HD, TB], MDT, name=f"a2a_in{h}")
                      for h in range(GH)]
            a2a_out = [dramp.tile([N_CORES, HD, TB], MDT, name=f"a2a_out{h}")
                       for h in range(GH)]

            def emit_attn(h, b):
                off = HD * h
                if True:
                    for qb in range(S // TB):
                        qcol = b * S + qb * TB
                        qsl = slice(qcol, qcol + TB)
                        pa = ps_at.tile([HD + 1, TB], F32, tag="at",
                                        name=f"pa{h}{b}{qb}")
                        exps = []
                        for kp in range(NP):
                            pssc = ps_sc.tile([128, 2 * TB], F32, tag="sc",
                                              name=f"pssc{h}{b}{qb}{kp}")
                            for i in range(2):
                                kcol = b * S + (2 * kp + i) * 128
                                nc.tensor.matmul(
                                    pssc[:, i * TB:(i + 1) * TB],
                                    KTp[h][:, kcol:kcol + 128],
                                    QT[:, qsl], start=True, stop=True)
                            ex = expp.tile([128, 2 * TB], MDT, tag="exp",
                                           name=f"ex{h}{b}{qb}{kp}")
                            nc.scalar.activation(ex[:], pssc[:], Act.Exp,
                                                 scale=0.125)
                            exps.append(ex)
                            if kp >= 1:
                                for i in range(2):
                                    kt = 2 * (kp - 1) + i
                                    nc.tensor.matmul(
                                        pa[:],
                                        VT[:, b * NKT + kt, h, :],
                                        exps[kp - 1][:, i * TB:(i + 1) * TB],
                                        start=(kt == 0), stop=False)
                        for i in range(2):
                            kt = 2 * (NP - 1) + i
                            nc.tensor.matmul(
                                pa[:], VT[:, b * NKT + kt, h, :],
                                exps[NP - 1][:, i * TB:(i + 1) * TB],
                                start=False, stop=(i == 1))
                        # normalize: attnT_h *= 1/den (broadcast over d)
                        dn = small.tile([1, TB], F32, tag="rc",
                                        name=f"dn{h}{b}{qb}")
                        nc.vector.tensor_copy(dn[:], pa[HD:HD + 1, :])
                        bc = small.tile([HD, TB], F32, tag="bc",
                                        name=f"bc{h}{b}{qb}")
                        nc.gpsimd.partition_broadcast(bc[:], dn[:])
                        rc = small.tile([HD, TB], F32, tag="rc2",
                                        name=f"rc{h}{b}{qb}")
                        nc.vector.reciprocal(rc[:], bc[:])
                        nc.vector.tensor_mul(
                            AT[off:off + HD, qsl], pa[0:HD, :], rc[:])

            def emit_a2a_half(h, b):
                off = HD * h
                nc.sync.dma_start(
                    a2a_in[h][4 * b:4 * b + 4, :, :].rearrange(
                        "j p n -> p j n"),
                    AT[off:off + HD, b * S:(b + 1) * S].rearrange(
                        "p (j n) -> p j n", j=4))

            def emit_a2a(h):
                off = HD * h
                nc.gpsimd.collective_compute(
                    "AllToAll",
                    mybir.AluOpType.bypass,
                    replica_groups=groups,
                    ins=[a2a_in[h][:]],
                    outs=[a2a_out[h][:]],
                )
                nc.sync.dma_start(
                    ao[HD * h:HD * h + HD, :, :],
                    a2a_out[h][:].rearrange("j p n -> p j n"))

            ao = state.tile([128, ECH, TB], MDT, name="ao")

            # ---- schedule: interleave attention between the two batches'
            # projections so ScalarE (exp) never idles; overlap A2A #1
            # and the Wo prefetch with head 1's attention.
            emit_proj(0)
            emit_proj(1)
            emit_attn(0, 0)
            emit_a2a_half(0, 0)
            emit_attn(0, 1)
            emit_a2a_half(0, 1)
            emit_a2a(0)
            nc.sync.dma_start(wo_sb[:], woT[:])
            nc.sync.dma_start(bo_sb[:], bo_d[:])
            emit_attn(1, 0)
            emit_a2a_half(1, 0)
            emit_attn(1, 1)
            emit_a2a_half(1, 1)
            emit_a2a(1)

            # ---- output projection over own 512-token slice ----
            for m in range(4):
                ot = opool.tile([128, D], F32, tag="ot", name=f"ot{m}")
                for fb in range(2):
                    fsl = slice(fb * 512, (fb + 1) * 512)
                    pso = ps_proj.tile([128, 512], F32, tag="pp",
                                       name=f"pso{m}_{fb}")
                    for nq in range(ECH):
                        nc.tensor.matmul(
                            pso[:], ao[:, nq, m * 128:(m + 1) * 128],
                            wo_sb[:, nq, fsl],
                            start=(nq == 0), stop=(nq == ECH - 1))
                    nc.vector.tensor_add(ot[:, fsl], pso[:], bo_sb[:, fsl])
                nc.sync.dma_start(out_d[m * 128:(m + 1) * 128, :], ot[:])

    nc.compile()
    return nc


def _mm_np_dtype():
    if MM_DTYPE == "bf16":
        import ml_dtypes
        return np.dtype(ml_dtypes.bfloat16)
    return np.float32


def _prep_inputs(Q_input, K_input, V_input, Wq, bq, Wk, bk, Wv, bv, Wo, bo):
    """Build the 8 per-core input maps (host-side sharding + transposes)."""
    f32 = np.float32
    mmdt = _mm_np_dtype()
    xT = {}
    for nm, x in (("xqT", Q_input), ("xkT", K_input), ("xvT", V_input)):
        x = np.asarray(x, f32)
        xT[nm] = np.ascontiguousarray(
            np.concatenate([x[b].T for b in range(B)], axis=1).astype(mmdt))
    Wq, Wk, Wv, Wo = (np.asarray(w, f32) for w in (Wq, Wk, Wv, Wo))
    bq, bk, bv, bo = (np.asarray(v, f32) for v in (bq, bk, bv, bo))

    def peF(wT):  # [D, F] -> [128, ECH, F] partition-major (fat descriptors)
        return np.ascontiguousarray(
            wT.reshape(ECH, 128, wT.shape[1]).transpose(1, 0, 2).astype(mmdt))

    woT_full = peF(Wo.T)
    bo_bc = np.ascontiguousarray(np.broadcast_to(bo, (128, D)))
    kmask = np.zeros((128, GH), f32)
    for h in range(GH):
        kmask[HD * h:HD * h + HD, h] = 1.0

    in_maps = []
    for c in range(N_CORES):
        hsl = slice(c * GF, (c + 1) * GF)
        in_maps.append({
            **xT,
            "wqT": peF(Wq[hsl, :].T),
            "wkT": peF(Wk[hsl, :].T),
            "wvT": peF(Wv[hsl, :].T),
            "woT": woT_full,
            "bq": np.ascontiguousarray(bq[hsl].reshape(128, 1)),
            "bk": np.ascontiguousarray(bk[hsl].reshape(128, 1)),
            "kmask": kmask,
            "kbm": np.ascontiguousarray(kmask * bk[hsl].reshape(128, 1)),
            "bv": np.ascontiguousarray(
                np.broadcast_to(np.tile(bv[hsl], 4), (128, TB))),
            "bo": bo_bc,
        })
    return in_maps


def kernel(**inputs):
    from concourse.bass_utils import run_bass_kernel_spmd

    if "nc" not in _CACHE:
        _CACHE["nc"] = _build()
    nc = _CACHE["nc"]

    in_maps = _prep_inputs(**inputs)
    res = run_bass_kernel_spmd(nc, in_maps, core_ids=list(range(N_CORES)))

    out = np.empty((B, S, D), np.float32)
    for c in range(N_CORES):
        b, j = divmod(c, S // TB)
        out[b, j * TB:(j + 1) * TB, :] = res.results[c]["out"]
    return out
